# revision 38
# baseline (speedup 1.0000x reference)
"""Trainium2 Bass kernel for nn_AttachmentPredictor (masked-row packed).

Only ~50% of sequence positions survive the mask; the reference zeroes the
rest. Host packs the masked-in rows of each core's batches contiguously
(batches load-balanced across cores), so the device processes ~4096 rows
instead of 8192. Per-row prep/child bias and the per-batch exp-sum
normalization are handled with small one-hot matmuls on the PE:

  stage1:  psum[j, r]  = sum_d wh[d, j] x[d, r]  (+ biasT one-hot matmul);
           fp8e4m3 DoubleRow by default (rel err ~1.6e-2 < 2e-2 gate)
  c1 = tanh(psum / S);  stage 2 the same with w0 (bf16)
  stage 3 row-major: psum[r, q] = sum_p c2[p, r] w1[p, q]; the scorer is a
           DVE multiply+reduce along q -> scoresT [128, 4] per block
  exp -> one-hot segsum matmuls accumulate per-batch sums
  phase 2: recip(sums) broadcast back to rows via one-hot matmuls, multiply,
  one DMA of packed scores; host scatters into the [B, S-2] zeros.
"""

import ml_dtypes
import numpy as np

import concourse.bass as bass
import concourse.mybir as mybir
import concourse.tile as tile
from concourse import bass_utils
from concourse.bass import ts

F32 = mybir.dt.float32
F32R = mybir.dt.float32r
BF16 = mybir.dt.bfloat16
F8E4 = mybir.dt.float8e4
AF = mybir.ActivationFunctionType
DR = mybir.MatmulPerfMode.DoubleRow

B, S, D, P = 256, 256, 1024, 512
NCORES = 8
BC = B // NCORES            # batches per core
KD = D // 128               # 8 k-tiles over D
KP = P // 128               # 4 k-tiles over P
EPS = 1e-7

OPTS = {
    "s1": "dr",      # "dr" (fp8e4 DoubleRow) | "bf16" | "f32r"
    "s2": "bf16",
    "s3": "bf16",
    "sc": "bf16",
    "bias_w": "bf16",  # wp/wc dtype for the bias projections
    "xr_bufs": 4,
    "c_bufs": 2,
    "ps_bufs": 5,
    "ps2_bufs": 3,
}

SX = 2.0    # fp8 quant scale for x
SW = 64.0   # fp8 quant scale for weights

_NPDT = {"dr": ml_dtypes.float8_e4m3, "bf16": ml_dtypes.bfloat16,
         "f32r": np.float32}
_BDT = {"dr": F8E4, "bf16": BF16, "f32r": F32R}


def _scales(kind):
    # (x_or_act_scale, w_scale) used when host-quantizing that stage's inputs
    if kind == "dr":
        return SX, SW
    return 1.0, 1.0


# ---------------------------------------------------------------------------
# walrus in this container accepts at most ONE sync wait per instruction;
# split extra waits onto preceding NoOps on the same engine.
def _split_waits(nc, maxw=1):
    ctr = 0
    for f in nc.m.functions:
        for blk in f.blocks:
            insts = blk.instructions
            newlist = []
            changed = False
            for inst in insts:
                si = inst.sync_info
                if si is not None and len(si.on_wait) > maxw:
                    waits = list(si.on_wait)
                    keep = waits[len(waits) - maxw:]
                    extra = waits[: len(waits) - maxw]
                    for j in range(0, len(extra), maxw):
                        ctr += 1
                        newlist.append(
                            mybir.InstNoOp(
                                name=f"waitsplit-{ctr}",
                                engine=inst.engine,
                                ins=[],
                                outs=[],
                                sync_info=mybir.SyncInfo(
                                    on_wait=extra[j: j + maxw], on_update=[]
                                ),
                            )
                        )
                    inst.sync_info = mybir.SyncInfo(
                        on_wait=keep, on_update=list(si.on_update)
                    )
                    changed = True
                newlist.append(inst)
            if changed:
                insts[:] = newlist


# ---------------------------------------------------------------------------
def _build(nrow, opts=None, reps=1):
    opts = dict(OPTS, **(opts or {}))
    nblk = nrow // 512
    d1, d2, d3, dsc = (_BDT[opts[k]] for k in ("s1", "s2", "s3", "sc"))
    dr1, dr2, dr3, drsc = (opts[k] == "dr" for k in ("s1", "s2", "s3", "sc"))
    sx1, sw1 = _scales(opts["s1"])
    _, sw2 = _scales(opts["s2"])
    _, sw3 = _scales(opts["s3"])
    _, swsc = _scales(opts["sc"])

    nc = bass.Bass("TRN2", target_bir_lowering=False, debug=False)

    dwb = _BDT[opts["bias_w"]]
    swb = SW if opts["bias_w"] == "dr" else 1.0

    xT_d = nc.dram_tensor("xT", [nblk, 128, KD * 512], d1,
                          kind="ExternalInput").ap()
    wh_d = nc.dram_tensor("wh", [128, KD * 512], d1, kind="ExternalInput").ap()
    xpc_d = nc.dram_tensor("xpc", [128, 2 * KD * BC], BF16,
                           kind="ExternalInput").ap()
    wpc_d = nc.dram_tensor("wpc", [128, 2 * KD * 512], dwb,
                           kind="ExternalInput").ap()
    assert d2 == d3, "w0/w1 share one dram blob"
    ww_d = nc.dram_tensor("ww", [128, 2 * KP * 512], d2,
                          kind="ExternalInput").ap()
    scb_d = nc.dram_tensor("scb", [128, 512], BF16, kind="ExternalInput").ap()
    ohb_d = nc.dram_tensor("ohb", [BC, nrow], BF16, kind="ExternalInput").ap()
    ohs_d = nc.dram_tensor("ohs", [128, nblk * 4 * BC], BF16,
                           kind="ExternalInput").ap()
    out_d = nc.dram_tensor("out", [128, nblk * 4], F32,
                           kind="ExternalOutput").ap()

    with tile.TileContext(nc) as tc:
        with (
            tc.tile_pool(name="consts", bufs=1) as consts,
            tc.tile_pool(name="ssb", bufs=3) as spool,
            tc.tile_pool(name="xr", bufs=opts["xr_bufs"]) as xpool,
            tc.tile_pool(name="acts", bufs=opts["c_bufs"]) as cpool,
            tc.tile_pool(name="epi", bufs=2) as epool,
            tc.tile_pool(name="expt", bufs=2) as xppool,
            tc.tile_pool(name="ps", bufs=opts["ps_bufs"], space="PSUM") as pspool,
            tc.tile_pool(name="ps2", bufs=opts["ps2_bufs"], space="PSUM") as ps2pool,
        ):
            # ---- x block prefetch starts immediately on the sync queue -----
            # two half-tiles per block so stage-1 can start on the first half
            xrs = {}
            hw = KD * 512 // 2

            def xr_fetch(blk):
                halves = []
                for h in range(2):
                    xh = xpool.tile([128, hw], d1, tag=f"xr{h}")
                    nc.sync.dma_start(xh[:], xT_d[blk, :, h * hw: (h + 1) * hw])
                    halves.append(xh)
                xrs[blk] = halves

            # ---- explicit DMA order on the single sync queue ---------------
            # the bias projections gate every activation, so their inputs go
            # first and the bias matmuls run at the very head of the PE
            # stream while x/wh arrive behind them. Few, large transfers:
            # each trigger costs ~0.6 us of serialized HWDGE time.
            ones_t = consts.tile([1, 1], F32, tag="ones")
            nc.vector.memset(ones_t[:], 1.0)
            xpc_t = consts.tile([128, 2 * KD * BC], BF16, tag="xpc")
            nc.sync.dma_start(xpc_t[:], xpc_d)
            wpc_t = consts.tile([128, 2 * KD * 512], dwb, tag="wpc")
            wpch = KD * 512
            for h in range(2):
                nc.sync.dma_start(wpc_t[:, h * wpch: (h + 1) * wpch],
                                  wpc_d[:, h * wpch: (h + 1) * wpch])
            wh_t = consts.tile([128, KD * 512], d1, tag="wh")
            half = KD * 512 // 2
            b0_halves = []
            for h in range(2):
                xh = xpool.tile([128, hw], d1, tag=f"xr{h}")
                nc.sync.dma_start(xh[:], xT_d[0, :, h * hw: (h + 1) * hw])
                b0_halves.append(xh)
                nc.sync.dma_start(wh_t[:, h * half: (h + 1) * half],
                                  wh_d[:, h * half: (h + 1) * half])
            xrs[0] = b0_halves
            whv = wh_t[:].rearrange("p (o r) -> p o r", r=512)
            ohb_t = consts.tile([BC, nrow], BF16, tag="ohb")
            nc.sync.dma_start(ohb_t[:], ohb_d)
            ww_t = consts.tile([128, 2 * KP * 512], d2, tag="ww")
            nc.sync.dma_start(ww_t[:, : KP * 512], ww_d[:, : KP * 512])
            xr_fetch(1)
            nc.sync.dma_start(ww_t[:, KP * 512:], ww_d[:, KP * 512:])
            w0v = ww_t[:, : KP * 512].rearrange("p (o r) -> p o r", r=512)
            w1v = ww_t[:, KP * 512:].rearrange("p (o r) -> p o r", r=512)
            scb_t = consts.tile([128, 512], BF16, tag="scb")
            nc.sync.dma_start(scb_t[:], scb_d)
            xr_fetch(2)
            ohs_t = consts.tile([128, nblk * 4 * BC], BF16, tag="ohs")
            nc.sync.dma_start(ohs_t[:], ohs_d)

            # ---- bias compute at the head of the PE stream ------------------
            psb = ps2pool.tile([BC, 512], F32, tag="ps2", name="psb")
            for i in range(2):
                for dk in range(KD):
                    g = i * KD + dk
                    nc.tensor.matmul(
                        psb[:],
                        xpc_t[:, g * BC: (g + 1) * BC],
                        wpc_t[:, g * 512: (g + 1) * 512],
                        start=(g == 0),
                        stop=(g == 2 * KD - 1),
                    )
            biasT = consts.tile([BC, 512], BF16, tag="biasT")
            nc.scalar.mul(biasT[:], psb[:], sx1 * sw1 / swb)

            # ---- main loop -------------------------------------------------
            for _rep in range(reps):
                sums = epool.tile([1, BC], F32, tag="sums", name=f"sums{_rep}")
                nc.vector.memset(sums[:], 0.0)
                expT_all = xppool.tile([128, nblk * 4], F32, tag="expT",
                                       name=f"expT{_rep}")
                for blk in range(nblk):
                    if blk not in xrs:
                        xr_fetch(blk)
                    xrA, xrB = xrs.pop(blk)
                    nxt = blk + 3
                    if _rep + 1 < reps and nxt >= nblk:
                        nxt -= nblk
                    if nxt < nblk and nxt not in xrs:
                        xr_fetch(nxt)
                    xvh = [xrA[:].rearrange("p (o r) -> p o r", r=512),
                           xrB[:].rearrange("p (o r) -> p o r", r=512)]
                    kdh = KD // 2

                    def xsl(o):
                        # single k-subtile o of the block's x
                        return xvh[o // kdh][:, o % kdh, :]

                    def xsl2(o):
                        # DR pair (2o, 2o+1); pairs never straddle halves
                        h, oo = (2 * o) // kdh, (2 * o) % kdh
                        return xvh[h][:, oo: oo + 2, :]

                    # stage 1 (+ bias) -> c1
                    c1 = cpool.tile([128, KP * 512], d2, tag="c1")
                    c1v = c1[:].rearrange("p (o r) -> p o r", r=512)

                    def s1_mains(ps, jt, o):
                        if dr1:
                            nc.tensor.matmul(
                                ps[:],
                                whv[:, 2 * o: 2 * o + 2, ts(jt, 128)],
                                xsl2(o),
                                start=(o == 0), stop=False,
                                perf_mode=DR,
                            )
                        else:
                            nc.tensor.matmul(
                                ps[:], whv[:, o, ts(jt, 128)], xsl(o),
                                start=(o == 0), stop=False,
                            )

                    def s1_bias_act(ps, jt):
                        nc.tensor.matmul(
                            ps[:],
                            biasT[:, ts(jt, 128)],
                            ohb_t[:, blk * 512: (blk + 1) * 512],
                            start=False, stop=True,
                        )
                        nc.scalar.activation(c1v[:, jt, :], ps[:], AF.Tanh,
                                             scale=1.0 / (sx1 * sw1))

                    ko1 = KD // 2 if dr1 else KD
                    for jt in range(KP):
                        ps = pspool.tile([128, 512], F32, tag="ps",
                                         name=f"ps1_{_rep}_{blk}_{jt}")
                        for o in range(ko1):
                            s1_mains(ps, jt, o)
                        s1_bias_act(ps, jt)

                    # stage 2 (feature-major)
                    c2 = cpool.tile([128, KP * 512], d3, tag="c2")
                    c2v = c2[:].rearrange("p (o r) -> p o r", r=512)
                    for qt in range(KP):
                        ps = pspool.tile([128, 512], F32, tag="ps",
                                         name=f"ps2_{_rep}_{blk}_{qt}")
                        if dr2:
                            for o in range(KP // 2):
                                nc.tensor.matmul(
                                    ps[:],
                                    w0v[:, 2 * o: 2 * o + 2, ts(qt, 128)],
                                    c1v[:, 2 * o: 2 * o + 2, :],
                                    start=(o == 0), stop=(o == KP // 2 - 1),
                                    perf_mode=DR,
                                )
                        else:
                            for o in range(KP):
                                nc.tensor.matmul(
                                    ps[:], w0v[:, o, ts(qt, 128)],
                                    c1v[:, o, :],
                                    start=(o == 0), stop=(o == KP - 1),
                                )
                        nc.scalar.activation(c2v[:, qt, :], ps[:], AF.Tanh,
                                             scale=1.0 / sw2)

                    # stage 3 row-major: psum [rows, q]; scorer as a DVE
                    # multiply-reduce along q -> scoresT [128, 4] directly in
                    # the transposed layout (no PE transposes, no scorer MMs)
                    c3 = cpool.tile([128, KP * 512], BF16, tag="c3")
                    c3v = c3[:].rearrange("p (t q) -> p t q", q=512)
                    scoresT = spool.tile([128, 4], F32, tag="scoresT",
                                         name=f"scT_{_rep}_{blk}")
                    for t in range(4):
                        ps = pspool.tile([128, 512], F32, tag="ps",
                                         name=f"ps3_{_rep}_{blk}_{t}")
                        for o in range(KP):
                            nc.tensor.matmul(
                                ps[:],
                                c2v[:, o, ts(t, 128)],
                                w1v[:, o, :],
                                start=(o == 0), stop=(o == KP - 1),
                            )
                        nc.scalar.activation(c3v[:, t, :], ps[:], AF.Tanh,
                                             scale=1.0 / sw3)
                        prod = spool.tile([128, 512], BF16, tag="ttr")
                        nc.vector.tensor_mul(prod[:], c3v[:, t, :], scb_t[:])
                        nc.vector.tensor_reduce(
                            scoresT[:, t: t + 1], prod[:],
                            axis=mybir.AxisListType.X,
                            op=mybir.AluOpType.add,
                        )
                    expT = expT_all[:, blk * 4: (blk + 1) * 4]
                    nc.scalar.activation(expT, scoresT[:], AF.Exp)
                    expTb = spool.tile([128, 4], BF16, tag="expTb",
                                       name=f"expTb_{_rep}_{blk}")
                    nc.vector.tensor_copy(expTb[:], expT)
                    pseg = ps2pool.tile([1, BC], F32, tag="ps2",
                                        name=f"pseg_{_rep}_{blk}")
                    for t in range(4):
                        nc.tensor.matmul(
                            pseg[:],
                            expTb[:, t: t + 1],
                            ohs_t[:, (blk * 4 + t) * BC: (blk * 4 + t + 1) * BC],
                            start=(t == 0), stop=(t == 3),
                        )
                    nc.vector.tensor_add(sums[:], sums[:], pseg[:])

                # ---- phase 2: normalize + output --------------------------
                nc.vector.tensor_scalar_add(sums[:], sums[:], EPS)
                recip = epool.tile([1, BC], F32, tag="recip",
                                   name=f"recip{_rep}")
                nc.vector.reciprocal(recip[:], sums[:])
                psr = ps2pool.tile([BC, 1], F32, tag="ps2", name=f"psr{_rep}")
                nc.tensor.matmul(psr[:], recip[:], ones_t[:],
                                 is_transpose=True)
                recipT = epool.tile([BC, 1], BF16, tag="recipT",
                                    name=f"recipT{_rep}")
                nc.vector.tensor_copy(recipT[:], psr[:])
                prr = ps2pool.tile([128, nblk * 4], F32, tag="ps2",
                                   name=f"prr_{_rep}")
                for blk in range(nblk):
                    for t in range(4):
                        nc.tensor.matmul(
                            prr[:, blk * 4 + t: blk * 4 + t + 1],
                            ohb_t[:, blk * 512 + t * 128: blk * 512 + (t + 1) * 128],
                            recipT[:],
                            start=True, stop=True,
                        )
                outv = spool.tile([128, nblk * 4], F32, tag="outv",
                                  name=f"outv_{_rep}")
                nc.vector.tensor_mul(outv[:], expT_all[:], prr[:])
                nc.sync.dma_start(out_d, outv[:])

    _split_waits(nc)
    return nc


# ---------------------------------------------------------------------------
def _host_prep(x, proj_head, proj_prep, proj_child, hidden_layers, scorer, mask,
               opts=None):
    opts = dict(OPTS, **(opts or {}))
    x = np.asarray(x, np.float32)
    mask = np.asarray(mask)
    head_mask = mask[:, : S - 2]
    counts = head_mask.sum(axis=1).astype(np.int64)  # [B]

    # balance batches across cores (LPT, capacity BC per core)
    order = np.argsort(-counts, kind="stable")
    core_batches = [[] for _ in range(NCORES)]
    core_rows = np.zeros(NCORES, np.int64)
    for b in order:
        cands = [c for c in range(NCORES) if len(core_batches[c]) < BC]
        c = min(cands, key=lambda c: core_rows[c])
        core_batches[c].append(int(b))
        core_rows[c] += counts[b]
    nrow = int(max(512, ((core_rows.max() + 511) // 512) * 512))
    nblk = nrow // 512

    np1, np2, np3, npsc = (_NPDT[opts[k]] for k in ("s1", "s2", "s3", "sc"))
    sx1, sw1 = _scales(opts["s1"])
    _, sw2 = _scales(opts["s2"])
    _, sw3 = _scales(opts["s3"])
    _, swsc = _scales(opts["sc"])

    wh = np.asarray(proj_head, np.float32)
    hl = np.asarray(hidden_layers, np.float32)
    sc = np.asarray(scorer, np.float32)

    # weight tiles [128, ktiles*512]: element (p, o*512+j) = W[o*128+p, j]*sw
    wh_pk = np.ascontiguousarray(
        (wh * sw1).reshape(KD, 128, P).transpose(1, 0, 2).astype(np1)
    ).reshape(128, KD * P)
    w0_pk = np.ascontiguousarray(
        (hl[0] * sw2).reshape(KP, 128, P).transpose(1, 0, 2).astype(np2)
    ).reshape(128, KP * P)
    w1_pk = np.ascontiguousarray(
        (hl[1] * sw3).reshape(KP, 128, P).transpose(1, 0, 2).astype(np3)
    ).reshape(128, KP * P)
    scb_pk = np.ascontiguousarray(
        np.broadcast_to(sc, (128, P)).astype(ml_dtypes.bfloat16)
    )  # [128, 512] scorer row replicated across partitions
    ww_pk = np.concatenate([w0_pk, w1_pk], axis=1)
    npwb = _NPDT[opts["bias_w"]]
    swb = SW if opts["bias_w"] == "dr" else 1.0
    wp32 = np.asarray(proj_prep, np.float32)
    wc32 = np.asarray(proj_child, np.float32)
    wpc_pk = np.concatenate(
        [
            np.ascontiguousarray(
                (w * swb).reshape(KD, 128, P).transpose(1, 0, 2).astype(npwb)
            ).reshape(128, KD * P)
            for w in (wp32, wc32)
        ],
        axis=1,
    )

    in_maps, scatter = [], []
    for c in range(NCORES):
        bs = core_batches[c]
        b_loc, s_idx, g_idx = [], [], []
        for i, gb in enumerate(bs):
            ss = np.nonzero(head_mask[gb])[0]
            b_loc.append(np.full(len(ss), i, np.int64))
            s_idx.append(ss)
            g_idx.append(np.full(len(ss), gb, np.int64))
        b_loc = np.concatenate(b_loc) if b_loc else np.zeros(0, np.int64)
        s_idx = np.concatenate(s_idx) if s_idx else np.zeros(0, np.int64)
        g_idx = np.concatenate(g_idx) if g_idx else np.zeros(0, np.int64)
        T = len(s_idx)

        xg = np.zeros((nrow, D), np.float32)
        xg[:T] = x[g_idx, s_idx]
        xT = np.ascontiguousarray(
            (xg * sx1).reshape(nblk, 512, KD, 128).transpose(0, 3, 2, 1)
            .astype(np1)
        ).reshape(nblk, 128, KD * 512)

        xb = x[np.asarray(bs, np.int64)]                     # [BC, S, D]
        xpc_pk = np.concatenate(
            [
                np.ascontiguousarray(
                    xb[:, s, :].T.reshape(KD, 128, BC).transpose(1, 0, 2)
                    .astype(ml_dtypes.bfloat16)
                ).reshape(128, KD * BC)
                for s in (S - 2, S - 1)
            ],
            axis=1,
        )

        ohb = np.zeros((BC, nrow), np.float32)
        ohb[b_loc, np.arange(T)] = 1.0
        ohs = np.ascontiguousarray(
            ohb.T.reshape(nblk, 4, 128, BC).transpose(2, 0, 1, 3)
        ).reshape(128, nblk * 4 * BC)

        in_maps.append({
            "xT": xT, "wh": wh_pk, "ww": ww_pk, "scb": scb_pk,
            "xpc": xpc_pk, "wpc": wpc_pk,
            "ohb": ohb.astype(ml_dtypes.bfloat16),
            "ohs": ohs.astype(ml_dtypes.bfloat16),
        })
        scatter.append((g_idx, s_idx))
    return in_maps, scatter, nrow


_NC_CACHE = {}


def _get_nc(nrow, opts=None, reps=1):
    key = (nrow, reps, tuple(sorted((dict(OPTS, **(opts or {}))).items())))
    if key not in _NC_CACHE:
        _NC_CACHE[key] = _build(nrow, opts=opts, reps=reps)
    return _NC_CACHE[key]


def kernel(x, proj_head, proj_prep, proj_child, hidden_layers, scorer, mask,
           opts=None):
    in_maps, scatter, nrow = _host_prep(
        x, proj_head, proj_prep, proj_child, hidden_layers, scorer, mask,
        opts=opts,
    )
    nc = _get_nc(nrow, opts=opts)
    res = bass_utils.run_bass_kernel_spmd(
        nc, in_maps, core_ids=list(range(NCORES))
    )
    out = np.zeros((B, S - 2), np.float32)
    for c in range(NCORES):
        vals = res.results[c]["out"]          # [128, nblk*4]
        nblk = vals.shape[1] // 4
        flat = vals.reshape(128, nblk, 4).transpose(1, 2, 0).reshape(-1)
        g_idx, s_idx = scatter[c]
        out[g_idx, s_idx] = flat[: len(g_idx)]
    return out


if __name__ == "__main__":
    rng = np.random.default_rng(0)
    x = rng.standard_normal((B, S, D)).astype(np.float32)
    u = lambda shp: rng.uniform(-0.05, 0.05, shp).astype(np.float32)
    inputs = dict(
        x=x, proj_head=u((D, P)), proj_prep=u((D, P)), proj_child=u((D, P)),
        hidden_layers=u((2, P, P)), scorer=u((P,)),
        mask=rng.integers(0, 2, (B, S)).astype(bool),
    )
    out = kernel(**inputs)
    print("kernel out", out.shape, out.dtype, out[:2, :4])


# revision 43
# speedup vs baseline: 1.2447x; 1.2447x over previous
"""Trainium2 Bass kernel for nn_AttachmentPredictor (masked-row packed).

Only ~50% of sequence positions survive the mask; the reference zeroes the
rest. Host packs the masked-in rows of each core's batches contiguously
(batches load-balanced across cores), so the device processes ~4096 rows
instead of 8192. Per-row prep/child bias and the per-batch exp-sum
normalization are handled with small one-hot matmuls on the PE:

  stage1:  psum[j, r]  = sum_d wh[d, j] x[d, r]  (+ biasT one-hot matmul);
           fp8e4m3 DoubleRow by default (rel err ~1.6e-2 < 2e-2 gate)
  c1 = tanh(psum / S);  stage 2 the same with w0 (bf16)
  stage 3 row-major: psum[r, q] = sum_p c2[p, r] w1[p, q]; the scorer is a
           DVE multiply+reduce along q -> scoresT [128, 4] per block
  exp -> one-hot segsum matmuls accumulate per-batch sums
  phase 2: recip(sums) broadcast back to rows via one-hot matmuls, multiply,
  one DMA of packed scores; host scatters into the [B, S-2] zeros.
"""

import ml_dtypes
import numpy as np

import concourse.bass as bass
import concourse.mybir as mybir
import concourse.tile as tile
from concourse import bass_utils
from concourse.bass import ts

F32 = mybir.dt.float32
F32R = mybir.dt.float32r
BF16 = mybir.dt.bfloat16
F8E4 = mybir.dt.float8e4
AF = mybir.ActivationFunctionType
DR = mybir.MatmulPerfMode.DoubleRow

B, S, D, P = 256, 256, 1024, 512
NCORES = 8
BC = B // NCORES            # batches per core
KD = D // 128               # 8 k-tiles over D
KP = P // 128               # 4 k-tiles over P
EPS = 1e-7

OPTS = {
    "s1": "dr",      # "dr" (fp8e4 DoubleRow) | "bf16" | "f32r"
    "s2": "bf16",
    "s3": "bf16",
    "sc": "bf16",
    "bias_w": "bf16",  # wp/wc dtype for the bias projections
    "xr_bufs": 4,
    "c_bufs": 2,
    "ps_bufs": 5,
    "ps2_bufs": 3,
}

SX = 2.0    # fp8 quant scale for x
SW = 64.0   # fp8 quant scale for weights

_NPDT = {"dr": ml_dtypes.float8_e4m3, "bf16": ml_dtypes.bfloat16,
         "f32r": np.float32}
_BDT = {"dr": F8E4, "bf16": BF16, "f32r": F32R}


def _scales(kind):
    # (x_or_act_scale, w_scale) used when host-quantizing that stage's inputs
    if kind == "dr":
        return SX, SW
    return 1.0, 1.0


# ---------------------------------------------------------------------------
# walrus in this container accepts at most ONE sync wait per instruction;
# split extra waits onto preceding NoOps on the same engine.
def _split_waits(nc, maxw=1):
    ctr = 0
    for f in nc.m.functions:
        for blk in f.blocks:
            insts = blk.instructions
            newlist = []
            changed = False
            for inst in insts:
                si = inst.sync_info
                if si is not None and len(si.on_wait) > maxw:
                    waits = list(si.on_wait)
                    keep = waits[len(waits) - maxw:]
                    extra = waits[: len(waits) - maxw]
                    for j in range(0, len(extra), maxw):
                        ctr += 1
                        newlist.append(
                            mybir.InstNoOp(
                                name=f"waitsplit-{ctr}",
                                engine=inst.engine,
                                ins=[],
                                outs=[],
                                sync_info=mybir.SyncInfo(
                                    on_wait=extra[j: j + maxw], on_update=[]
                                ),
                            )
                        )
                    inst.sync_info = mybir.SyncInfo(
                        on_wait=keep, on_update=list(si.on_update)
                    )
                    changed = True
                newlist.append(inst)
            if changed:
                insts[:] = newlist


# ---------------------------------------------------------------------------
def _build(nrow, opts=None, reps=1):
    opts = dict(OPTS, **(opts or {}))
    nblk = nrow // 512
    d1, d2, d3, dsc = (_BDT[opts[k]] for k in ("s1", "s2", "s3", "sc"))
    dr1, dr2, dr3, drsc = (opts[k] == "dr" for k in ("s1", "s2", "s3", "sc"))
    sx1, sw1 = _scales(opts["s1"])
    _, sw2 = _scales(opts["s2"])
    _, sw3 = _scales(opts["s3"])
    _, swsc = _scales(opts["sc"])

    nc = bass.Bass("TRN2", target_bir_lowering=False, debug=False)

    dwb = _BDT[opts["bias_w"]]
    swb = SW if opts["bias_w"] == "dr" else 1.0

    xT_d = nc.dram_tensor("xT", [nblk, 128, KD * 512], d1,
                          kind="ExternalInput").ap()
    wh_d = nc.dram_tensor("wh", [128, KD * 512], d1, kind="ExternalInput").ap()
    xpc_d = nc.dram_tensor("xpc", [128, 2 * KD * BC], BF16,
                           kind="ExternalInput").ap()
    wpc_d = nc.dram_tensor("wpc", [128, 2 * KD * 512], dwb,
                           kind="ExternalInput").ap()
    assert d2 == d3, "w0/w1 share one dram blob"
    ww_d = nc.dram_tensor("ww", [128, 2 * KP * 512], d2,
                          kind="ExternalInput").ap()
    scb_d = nc.dram_tensor("scb", [128, 512], BF16, kind="ExternalInput").ap()
    ohb_d = nc.dram_tensor("ohb", [BC, nrow], BF16, kind="ExternalInput").ap()
    ohs_d = nc.dram_tensor("ohs", [128, nblk * 4 * BC], BF16,
                           kind="ExternalInput").ap()
    out_d = nc.dram_tensor("out", [128, nblk * 4], F32,
                           kind="ExternalOutput").ap()

    with tile.TileContext(nc) as tc:
        with (
            tc.tile_pool(name="consts", bufs=1) as consts,
            tc.tile_pool(name="ssb", bufs=3) as spool,
            tc.tile_pool(name="xr", bufs=opts["xr_bufs"]) as xpool,
            tc.tile_pool(name="acts", bufs=opts["c_bufs"]) as cpool,
            tc.tile_pool(name="epi", bufs=2) as epool,
            tc.tile_pool(name="expt", bufs=2) as xppool,
            tc.tile_pool(name="ps", bufs=opts["ps_bufs"], space="PSUM") as pspool,
            tc.tile_pool(name="ps2", bufs=opts["ps2_bufs"], space="PSUM") as ps2pool,
        ):
            # ---- x block prefetch starts immediately on the sync queue -----
            # two half-tiles per block so stage-1 can start on the first half
            xrs = {}
            hw = KD * 512 // 2

            def xr_fetch(blk):
                halves = []
                for h in range(2):
                    xh = xpool.tile([128, hw], d1, tag=f"xr{h}")
                    nc.sync.dma_start(xh[:], xT_d[blk, :, h * hw: (h + 1) * hw])
                    halves.append(xh)
                xrs[blk] = halves

            # ---- explicit DMA order on the single sync queue ---------------
            # the bias projections gate every activation, so their inputs go
            # first and the bias matmuls run at the very head of the PE
            # stream while x/wh arrive behind them. Few, large transfers:
            # each trigger costs ~0.6 us of serialized HWDGE time.
            ones_t = consts.tile([1, 1], F32, tag="ones")
            nc.vector.memset(ones_t[:], 1.0)
            xpc_t = consts.tile([128, 2 * KD * BC], BF16, tag="xpc")
            nc.sync.dma_start(xpc_t[:], xpc_d)
            wpc_t = consts.tile([128, 2 * KD * 512], dwb, tag="wpc")
            wpcq = KD * 512 // 2
            for h in range(4):
                nc.sync.dma_start(wpc_t[:, h * wpcq: (h + 1) * wpcq],
                                  wpc_d[:, h * wpcq: (h + 1) * wpcq])
            wh_t = consts.tile([128, KD * 512], d1, tag="wh")
            half = KD * 512 // 2
            b0_halves = []
            for h in range(2):
                xh = xpool.tile([128, hw], d1, tag=f"xr{h}")
                nc.sync.dma_start(xh[:], xT_d[0, :, h * hw: (h + 1) * hw])
                b0_halves.append(xh)
                nc.sync.dma_start(wh_t[:, h * half: (h + 1) * half],
                                  wh_d[:, h * half: (h + 1) * half])
            xrs[0] = b0_halves
            whv = wh_t[:].rearrange("p (o r) -> p o r", r=512)
            ohb_t = consts.tile([BC, nrow], BF16, tag="ohb")
            nc.sync.dma_start(ohb_t[:], ohb_d)
            ww_t = consts.tile([128, 2 * KP * 512], d2, tag="ww")
            nc.sync.dma_start(ww_t[:, : KP * 512], ww_d[:, : KP * 512])
            xr_fetch(1)
            nc.sync.dma_start(ww_t[:, KP * 512:], ww_d[:, KP * 512:])
            w0v = ww_t[:, : KP * 512].rearrange("p (o r) -> p o r", r=512)
            w1v = ww_t[:, KP * 512:].rearrange("p (o r) -> p o r", r=512)
            scb_t = consts.tile([128, 512], BF16, tag="scb")
            nc.sync.dma_start(scb_t[:], scb_d)
            xr_fetch(2)
            ohs_t = consts.tile([128, nblk * 4 * BC], BF16, tag="ohs")
            nc.sync.dma_start(ohs_t[:], ohs_d)

            # ---- bias compute at the head of the PE stream ------------------
            psb = ps2pool.tile([BC, 512], F32, tag="ps2", name="psb")
            for i in range(2):
                for dk in range(KD):
                    g = i * KD + dk
                    nc.tensor.matmul(
                        psb[:],
                        xpc_t[:, g * BC: (g + 1) * BC],
                        wpc_t[:, g * 512: (g + 1) * 512],
                        start=(g == 0),
                        stop=(g == 2 * KD - 1),
                    )
            biasT = consts.tile([BC, 512], BF16, tag="biasT")
            nc.scalar.mul(biasT[:], psb[:], sx1 * sw1 / swb)

            # ---- main loop -------------------------------------------------
            for _rep in range(reps):
                sums = epool.tile([1, BC], F32, tag="sums", name=f"sums{_rep}")
                nc.vector.memset(sums[:], 0.0)
                expT_all = xppool.tile([128, nblk * 4], F32, tag="expT",
                                       name=f"expT{_rep}")
                for blk in range(nblk):
                    if blk not in xrs:
                        xr_fetch(blk)
                    xrA, xrB = xrs.pop(blk)
                    nxt = blk + 3
                    if _rep + 1 < reps and nxt >= nblk:
                        nxt -= nblk
                    if nxt < nblk and nxt not in xrs:
                        xr_fetch(nxt)
                    xvh = [xrA[:].rearrange("p (o r) -> p o r", r=512),
                           xrB[:].rearrange("p (o r) -> p o r", r=512)]
                    kdh = KD // 2

                    def xsl(o):
                        # single k-subtile o of the block's x
                        return xvh[o // kdh][:, o % kdh, :]

                    def xsl2(o):
                        # DR pair (2o, 2o+1); pairs never straddle halves
                        h, oo = (2 * o) // kdh, (2 * o) % kdh
                        return xvh[h][:, oo: oo + 2, :]

                    # stage 1 (+ bias) -> c1
                    c1 = cpool.tile([128, KP * 512], d2, tag="c1")
                    c1v = c1[:].rearrange("p (o r) -> p o r", r=512)

                    def s1_mains(ps, jt, o):
                        if dr1:
                            nc.tensor.matmul(
                                ps[:],
                                whv[:, 2 * o: 2 * o + 2, ts(jt, 128)],
                                xsl2(o),
                                start=(o == 0), stop=False,
                                perf_mode=DR,
                            )
                        else:
                            nc.tensor.matmul(
                                ps[:], whv[:, o, ts(jt, 128)], xsl(o),
                                start=(o == 0), stop=False,
                            )

                    def s1_bias_act(ps, jt):
                        nc.tensor.matmul(
                            ps[:],
                            biasT[:, ts(jt, 128)],
                            ohb_t[:, blk * 512: (blk + 1) * 512],
                            start=False, stop=True,
                        )
                        nc.scalar.activation(c1v[:, jt, :], ps[:], AF.Tanh,
                                             scale=1.0 / (sx1 * sw1))

                    ko1 = KD // 2 if dr1 else KD
                    for jt in range(KP):
                        ps = pspool.tile([128, 512], F32, tag="ps",
                                         name=f"ps1_{_rep}_{blk}_{jt}")
                        for o in range(ko1):
                            s1_mains(ps, jt, o)
                        s1_bias_act(ps, jt)

                    # stage 2 (feature-major)
                    c2 = cpool.tile([128, KP * 512], d3, tag="c2")
                    c2v = c2[:].rearrange("p (o r) -> p o r", r=512)
                    for qt in range(KP):
                        ps = pspool.tile([128, 512], F32, tag="ps",
                                         name=f"ps2_{_rep}_{blk}_{qt}")
                        if dr2:
                            for o in range(KP // 2):
                                nc.tensor.matmul(
                                    ps[:],
                                    w0v[:, 2 * o: 2 * o + 2, ts(qt, 128)],
                                    c1v[:, 2 * o: 2 * o + 2, :],
                                    start=(o == 0), stop=(o == KP // 2 - 1),
                                    perf_mode=DR,
                                )
                        else:
                            for o in range(KP):
                                nc.tensor.matmul(
                                    ps[:], w0v[:, o, ts(qt, 128)],
                                    c1v[:, o, :],
                                    start=(o == 0), stop=(o == KP - 1),
                                )
                        nc.scalar.activation(c2v[:, qt, :], ps[:], AF.Tanh,
                                             scale=1.0 / sw2)

                    # stage 3 row-major: psum [rows, q]; scorer as a DVE
                    # multiply-reduce along q -> scoresT [128, 4] directly in
                    # the transposed layout (no PE transposes, no scorer MMs)
                    c3 = cpool.tile([128, KP * 512], BF16, tag="c3")
                    c3v = c3[:].rearrange("p (t q) -> p t q", q=512)
                    scoresT = spool.tile([128, 4], F32, tag="scoresT",
                                         name=f"scT_{_rep}_{blk}")
                    expTb = spool.tile([128, 4], BF16, tag="expTb",
                                       name=f"expTb_{_rep}_{blk}")
                    pseg = ps2pool.tile([1, BC], F32, tag="ps2",
                                        name=f"pseg_{_rep}_{blk}")
                    for t in range(4):
                        ps = pspool.tile([128, 512], F32, tag="ps",
                                         name=f"ps3_{_rep}_{blk}_{t}")
                        for o in range(KP):
                            nc.tensor.matmul(
                                ps[:],
                                c2v[:, o, ts(t, 128)],
                                w1v[:, o, :],
                                start=(o == 0), stop=(o == KP - 1),
                            )
                        nc.scalar.activation(c3v[:, t, :], ps[:], AF.Tanh,
                                             scale=1.0 / sw3)
                        prod = spool.tile([128, 512], BF16, tag="ttr")
                        nc.vector.tensor_mul(prod[:], c3v[:, t, :], scb_t[:])
                        nc.vector.tensor_reduce(
                            scoresT[:, t: t + 1], prod[:],
                            axis=mybir.AxisListType.X,
                            op=mybir.AluOpType.add,
                        )
                        # per-t exp/cast/segsum so the PE segsum fires as soon
                        # as this column's reduce lands (shortens the tail)
                        nc.scalar.activation(
                            expT_all[:, blk * 4 + t: blk * 4 + t + 1],
                            scoresT[:, t: t + 1], AF.Exp,
                        )
                        nc.vector.tensor_copy(
                            expTb[:, t: t + 1],
                            expT_all[:, blk * 4 + t: blk * 4 + t + 1],
                        )
                        nc.tensor.matmul(
                            pseg[:],
                            expTb[:, t: t + 1],
                            ohs_t[:, (blk * 4 + t) * BC: (blk * 4 + t + 1) * BC],
                            start=(t == 0), stop=(t == 3),
                        )
                    nc.vector.tensor_add(sums[:], sums[:], pseg[:])

                # ---- phase 2: normalize + output --------------------------
                nc.vector.tensor_scalar_add(sums[:], sums[:], EPS)
                recip = epool.tile([1, BC], F32, tag="recip",
                                   name=f"recip{_rep}")
                nc.vector.reciprocal(recip[:], sums[:])
                psr = ps2pool.tile([BC, 1], F32, tag="ps2", name=f"psr{_rep}")
                nc.tensor.matmul(psr[:], recip[:], ones_t[:],
                                 is_transpose=True)
                recipT = epool.tile([BC, 1], BF16, tag="recipT",
                                    name=f"recipT{_rep}")
                nc.vector.tensor_copy(recipT[:], psr[:])
                prr = ps2pool.tile([128, nblk * 4], F32, tag="ps2",
                                   name=f"prr_{_rep}")
                for blk in range(nblk):
                    for t in range(4):
                        nc.tensor.matmul(
                            prr[:, blk * 4 + t: blk * 4 + t + 1],
                            ohb_t[:, blk * 512 + t * 128: blk * 512 + (t + 1) * 128],
                            recipT[:],
                            start=True, stop=True,
                        )
                outv = spool.tile([128, nblk * 4], F32, tag="outv",
                                  name=f"outv_{_rep}")
                nc.vector.tensor_mul(outv[:], expT_all[:], prr[:])
                nc.sync.dma_start(out_d, outv[:])

    _split_waits(nc)
    return nc


# ---------------------------------------------------------------------------
def _host_prep(x, proj_head, proj_prep, proj_child, hidden_layers, scorer, mask,
               opts=None):
    opts = dict(OPTS, **(opts or {}))
    x = np.asarray(x, np.float32)
    mask = np.asarray(mask)
    head_mask = mask[:, : S - 2]
    counts = head_mask.sum(axis=1).astype(np.int64)  # [B]

    # balance batches across cores (LPT, capacity BC per core)
    order = np.argsort(-counts, kind="stable")
    core_batches = [[] for _ in range(NCORES)]
    core_rows = np.zeros(NCORES, np.int64)
    for b in order:
        cands = [c for c in range(NCORES) if len(core_batches[c]) < BC]
        c = min(cands, key=lambda c: core_rows[c])
        core_batches[c].append(int(b))
        core_rows[c] += counts[b]
    nrow = int(max(512, ((core_rows.max() + 511) // 512) * 512))
    nblk = nrow // 512

    np1, np2, np3, npsc = (_NPDT[opts[k]] for k in ("s1", "s2", "s3", "sc"))
    sx1, sw1 = _scales(opts["s1"])
    _, sw2 = _scales(opts["s2"])
    _, sw3 = _scales(opts["s3"])
    _, swsc = _scales(opts["sc"])

    wh = np.asarray(proj_head, np.float32)
    hl = np.asarray(hidden_layers, np.float32)
    sc = np.asarray(scorer, np.float32)

    # weight tiles [128, ktiles*512]: element (p, o*512+j) = W[o*128+p, j]*sw
    wh_pk = np.ascontiguousarray(
        (wh * sw1).reshape(KD, 128, P).transpose(1, 0, 2).astype(np1)
    ).reshape(128, KD * P)
    w0_pk = np.ascontiguousarray(
        (hl[0] * sw2).reshape(KP, 128, P).transpose(1, 0, 2).astype(np2)
    ).reshape(128, KP * P)
    w1_pk = np.ascontiguousarray(
        (hl[1] * sw3).reshape(KP, 128, P).transpose(1, 0, 2).astype(np3)
    ).reshape(128, KP * P)
    scb_pk = np.ascontiguousarray(
        np.broadcast_to(sc, (128, P)).astype(ml_dtypes.bfloat16)
    )  # [128, 512] scorer row replicated across partitions
    ww_pk = np.concatenate([w0_pk, w1_pk], axis=1)
    npwb = _NPDT[opts["bias_w"]]
    swb = SW if opts["bias_w"] == "dr" else 1.0
    wp32 = np.asarray(proj_prep, np.float32)
    wc32 = np.asarray(proj_child, np.float32)
    wpc_pk = np.concatenate(
        [
            np.ascontiguousarray(
                (w * swb).reshape(KD, 128, P).transpose(1, 0, 2).astype(npwb)
            ).reshape(128, KD * P)
            for w in (wp32, wc32)
        ],
        axis=1,
    )

    in_maps, scatter = [], []
    for c in range(NCORES):
        bs = core_batches[c]
        b_loc, s_idx, g_idx = [], [], []
        for i, gb in enumerate(bs):
            ss = np.nonzero(head_mask[gb])[0]
            b_loc.append(np.full(len(ss), i, np.int64))
            s_idx.append(ss)
            g_idx.append(np.full(len(ss), gb, np.int64))
        b_loc = np.concatenate(b_loc) if b_loc else np.zeros(0, np.int64)
        s_idx = np.concatenate(s_idx) if s_idx else np.zeros(0, np.int64)
        g_idx = np.concatenate(g_idx) if g_idx else np.zeros(0, np.int64)
        T = len(s_idx)

        xg = np.zeros((nrow, D), np.float32)
        xg[:T] = x[g_idx, s_idx]
        xT = np.ascontiguousarray(
            (xg * sx1).reshape(nblk, 512, KD, 128).transpose(0, 3, 2, 1)
            .astype(np1)
        ).reshape(nblk, 128, KD * 512)

        xb = x[np.asarray(bs, np.int64)]                     # [BC, S, D]
        xpc_pk = np.concatenate(
            [
                np.ascontiguousarray(
                    xb[:, s, :].T.reshape(KD, 128, BC).transpose(1, 0, 2)
                    .astype(ml_dtypes.bfloat16)
                ).reshape(128, KD * BC)
                for s in (S - 2, S - 1)
            ],
            axis=1,
        )

        ohb = np.zeros((BC, nrow), np.float32)
        ohb[b_loc, np.arange(T)] = 1.0
        ohs = np.ascontiguousarray(
            ohb.T.reshape(nblk, 4, 128, BC).transpose(2, 0, 1, 3)
        ).reshape(128, nblk * 4 * BC)

        in_maps.append({
            "xT": xT, "wh": wh_pk, "ww": ww_pk, "scb": scb_pk,
            "xpc": xpc_pk, "wpc": wpc_pk,
            "ohb": ohb.astype(ml_dtypes.bfloat16),
            "ohs": ohs.astype(ml_dtypes.bfloat16),
        })
        scatter.append((g_idx, s_idx))
    return in_maps, scatter, nrow


_NC_CACHE = {}


def _get_nc(nrow, opts=None, reps=1):
    key = (nrow, reps, tuple(sorted((dict(OPTS, **(opts or {}))).items())))
    if key not in _NC_CACHE:
        _NC_CACHE[key] = _build(nrow, opts=opts, reps=reps)
    return _NC_CACHE[key]


def kernel(x, proj_head, proj_prep, proj_child, hidden_layers, scorer, mask,
           opts=None):
    in_maps, scatter, nrow = _host_prep(
        x, proj_head, proj_prep, proj_child, hidden_layers, scorer, mask,
        opts=opts,
    )
    nc = _get_nc(nrow, opts=opts)
    res = bass_utils.run_bass_kernel_spmd(
        nc, in_maps, core_ids=list(range(NCORES))
    )
    out = np.zeros((B, S - 2), np.float32)
    for c in range(NCORES):
        vals = res.results[c]["out"]          # [128, nblk*4]
        nblk = vals.shape[1] // 4
        flat = vals.reshape(128, nblk, 4).transpose(1, 2, 0).reshape(-1)
        g_idx, s_idx = scatter[c]
        out[g_idx, s_idx] = flat[: len(g_idx)]
    return out


if __name__ == "__main__":
    rng = np.random.default_rng(0)
    x = rng.standard_normal((B, S, D)).astype(np.float32)
    u = lambda shp: rng.uniform(-0.05, 0.05, shp).astype(np.float32)
    inputs = dict(
        x=x, proj_head=u((D, P)), proj_prep=u((D, P)), proj_child=u((D, P)),
        hidden_layers=u((2, P, P)), scorer=u((P,)),
        mask=rng.integers(0, 2, (B, S)).astype(bool),
    )
    out = kernel(**inputs)
    print("kernel out", out.shape, out.dtype, out[:2, :4])
